# revision 31
# baseline (speedup 1.0000x reference)
"""Trainium2 Bass kernel for hierarchical softmax tree posterior (HNet.predict).

Math: per internal node i (level-order, children 2i+1/2i+2), softmax over 2
children of Linear(x). Path probabilities multiply down a depth-12 complete
binary tree; output p [B, 4096] leaf posteriors.

Key identities / layout tricks:
  softmax([l0,l1])[0] = sigmoid(l0-l1): only the logit DIFFERENCE matters,
  so one [B,65] @ [65,4095] matmul (bias folded as 65th row) gives all d.
  sigma(-d) = 1 - sigma(d): each tree level is mul + sub on VectorE.

  CONCAT layout: v_{l+1} = [v_l * s_l || v_l - v_l * s_l] keeps every DVE
  operand dense step-1, which (with fp16) enables the 2x_1p DVE perf mode:
  2 elem/cycle/lane vs 1 for fp32 or strided/interleaved child pairs.
  The price: the leaf axis comes out BIT-REVERSED (each level's branch bit
  becomes the MSB of the position index). The host un-permutes columns at
  gather time (and the tree-node columns of W are pre-permuted to match).

  fp16 everywhere after PSUM: halves DVE time (2x mode) AND halves the
  dominant output DMA traffic (16.8MB -> 8.4MB per core). A 2^14 scale is
  injected at the root so path products stay in fp16 normal range; host
  multiplies by 2^-14 (exact) after converting to f32.

Schedule: per-tile (128 rows) pipeline matmul -> sigmoid -> tree -> DMA so
output DMA overlaps compute from the first tile on. Levels 0..7 are batched
across tiles in two halves of 4 (amortizes DVE op overhead without gating
tile 0 on all 8 phase-A matmuls); the half-2 shallow tree is emitted after
tile 1 so it stays off the DVE critical path during ramp-up.

Sharding: batch B=8192 split across 8 cores (1024 rows each); tree params
replicated.
"""

import contextlib

import numpy as np

import concourse.bacc as bacc
import concourse.mybir as mybir
import concourse.tile as tile
from concourse.bass_utils import run_bass_kernel_spmd

B, D = 8192, 64
NODES = 4095          # internal nodes, level-order
LEAVES = 4096
DEPTH = 12
NCORES = 8
BLOC = B // NCORES    # 1024 rows per core
KA = D + 1            # contraction dim incl. bias row
NBT = BLOC // 128     # 8 batch tiles of 128 rows

C_SCALE = 2.0 ** 14   # root scale keeping fp16 path products normal

F32 = mybir.dt.float32
F16 = mybir.dt.float16
# float32r runs the PE at 1 cyc/row (vs 4 for exact fp32); DRAM inputs are
# declared float32r directly (same bytes as f32) so no on-device cast.
MM_DT = mybir.dt.float32r

# Column split of the per-tile matmul across two PSUM tiles (levels 8..11;
# levels 0..7 = cols 0..254 are matmul'd in phase A).
PS1_LO, PS1_HI = 255, 2303    # 2048 cols: 4x512 chunks
PS2_LO, PS2_HI = 2303, 4095   # 1792 cols: 512,512,512,256 chunks


def _build(reps=1):
    nc = bacc.Bacc("TRN2", target_bir_lowering=False, debug=False, num_devices=NCORES)
    wdt = nc.dram_tensor("wdt", [KA, LEAVES], MM_DT, kind="ExternalInput")
    xt = nc.dram_tensor("xt", [KA, BLOC], MM_DT, kind="ExternalInput")
    out = nc.dram_tensor("out", [BLOC, LEAVES], F16, kind="ExternalOutput")

    with tile.TileContext(nc) as tc:
        with (
            tc.tile_pool(name="const", bufs=1) as const,
            tc.tile_pool(name="pa", bufs=1) as pa,
            tc.tile_pool(name="pb", bufs=2) as pb,
            tc.tile_pool(name="ps", bufs=2, space="PSUM") as psp,
        ):
            wdt_r = const.tile([KA, LEAVES], MM_DT, tag="wdt_r")
            xt_r = const.tile([KA, BLOC], MM_DT, tag="xt_r")
            warm = const.tile([KA, 640], MM_DT, tag="warm")
            # memset of a float32r AP fails the neuronxcc ISA check; same bits
            # as f32 zero via a bitcast view.
            nc.vector.memset(warm[:].bitcast(F32), 0.0)
            # phase-A weight cols + tile 0-3 xt first: they gate the pipeline
            nc.sync.dma_start(out=wdt_r[:, 0:256], in_=wdt[:, 0:256])
            nc.sync.dma_start(out=xt_r[:, 0:512], in_=xt[:, 0:512])
            nc.sync.dma_start(out=xt_r[:, 512:BLOC], in_=xt[:, 512:BLOC])
            nc.sync.dma_start(out=wdt_r[:, 256:LEAVES], in_=wdt[:, 256:LEAVES])

            loop = tc.For_i(0, reps, 1) if reps > 1 else contextlib.nullcontext()
            with loop:
                _emit_body(nc, tc, pa, pb, psp, wdt_r, xt_r, out, warm=warm)

    nc.compile()
    return nc


def _phase_a_mm(nc, psp, wdt_r, xt_r, half, ncols, warm=None):
    """Matmul of tree cols 0..ncols-1 for tiles half*4..half*4+3, packed at
    ncols spacing in one 4-bank PSUM tile. The top pad col (a next-level
    node, recomputed per tile later) keeps N a multiple of 256: f32r matmuls
    with a moving dim under 256 pay a 4x per-row penalty."""
    ps = psp.tile([128, 4 * ncols], F32, tag="ps")
    if warm is not None:
        # PE pstate warm-up: the Tensor engine runs ~2-4x slower until it has
        # been continuously busy ~3us. Run throwaway matmuls on a memset tile
        # while the input DMAs are in flight so the real matmuls (behind these
        # in the in-order PE queue) issue at full clock. Results land in PSUM
        # cols 512:1024 and are overwritten by the real matmuls behind them
        # in the same queue before sigma reads the tile.
        for _ in range(6):
            nc.tensor.matmul(ps[:, 512:1024], warm[:, 0:128], warm[:, 128:640],
                             start=True, stop=True)
    for i in range(4):
        bt = half * 4 + i
        nc.tensor.matmul(
            ps[:, i * ncols:(i + 1) * ncols],
            xt_r[:, bt * 128:(bt + 1) * 128],
            wdt_r[:, 0:ncols],
            start=True, stop=True,
        )
    return ps


def _phase_a_tree(nc, s_A, vA, vB, half, depth):
    """Levels 0..depth-1 batched over 4 tiles: v_depth half [128, 4, 2**depth]."""
    MUL = mybir.AluOpType.mult
    ADD = mybir.AluOpType.add
    g = slice(half * 4, (half + 1) * 4)
    # level 0: v1 = [C*s0, C - C*s0]
    nc.vector.tensor_scalar_mul(vA[:, g, 0:1], s_A[:, g, 0:1], C_SCALE)
    nc.vector.tensor_scalar(vA[:, g, 1:2], s_A[:, g, 0:1], -C_SCALE, C_SCALE, MUL, ADD)
    cur, nxt = vA, vB
    for lvl in range(1, depth):
        n = 1 << lvl
        off = n - 1
        nc.vector.tensor_mul(nxt[:, g, 0:n], cur[:, g, 0:n], s_A[:, g, off:off + n])
        nc.vector.tensor_sub(nxt[:, g, n:2 * n], cur[:, g, 0:n], nxt[:, g, 0:n])
        cur, nxt = nxt, cur
    return cur


def _tile_deep(nc, psp, pb, wdt_r, xt_r, out, v8, bt, fine_tail=False,
               defer=False):
    """Per batch tile: matmul cols 255..4094, sigmoid, levels 8..11, DMA.

    fine_tail splits the last sub+DMA into 1024-col pieces so the kernel's
    final DMA is 728ns instead of 1456ns (only worth it on the last tile).
    defer returns the DVE/DMA stage as a list of closures (matmul+sigma are
    emitted immediately) so two tiles can be op-interleaved: each op's
    predecessor is then 2 queue slots back and the ~95ns dependency-ack gap
    hides behind the other tile's op.
    """
    SIG = mybir.ActivationFunctionType.Sigmoid
    xs = xt_r[:, bt * 128:(bt + 1) * 128]
    ps1 = psp.tile([128, 2048], F32, tag="ps")   # cols 255..2302
    for c in range(4):
        nc.tensor.matmul(
            ps1[:, c * 512:(c + 1) * 512],
            xs, wdt_r[:, PS1_LO + c * 512:PS1_LO + (c + 1) * 512],
            start=True, stop=True,
        )
    ps2 = psp.tile([128, 1792], F32, tag="ps")   # cols 2303..4094
    for c in range(4):
        w0 = PS2_LO + c * 512
        w1 = min(w0 + 512, PS2_HI)
        nc.tensor.matmul(
            ps2[:, c * 512:c * 512 + (w1 - w0)],
            xs, wdt_r[:, w0:w1],
            start=True, stop=True,
        )
    # sigma of cols 255..4094, split so level 8 unblocks early
    s_B = pb.tile([128, 3840], F16, tag="sB")
    nc.scalar.activation(out=s_B[:, 0:768], in_=ps1[:, 0:768], func=SIG)
    nc.scalar.activation(out=s_B[:, 768:2048], in_=ps1[:, 768:2048], func=SIG)
    nc.scalar.activation(out=s_B[:, 2048:3840], in_=ps2[:], func=SIG)

    v9 = pb.tile([128, 512], F16, tag="v9")
    v10 = pb.tile([128, 1024], F16, tag="v10")
    v11 = pb.tile([128, 2048], F16, tag="v11")
    ot = pb.tile([128, 4096], F16, tag="out")
    rows = out[bt * 128:(bt + 1) * 128, :]
    ops = [
        lambda: nc.vector.tensor_mul(v9[:, 0:256], v8[:, bt, 0:256], s_B[:, 0:256]),
        lambda: nc.vector.tensor_sub(v9[:, 256:512], v8[:, bt, 0:256], v9[:, 0:256]),
        lambda: nc.vector.tensor_mul(v10[:, 0:512], v9[:], s_B[:, 256:768]),
        lambda: nc.vector.tensor_sub(v10[:, 512:1024], v9[:], v10[:, 0:512]),
        lambda: nc.vector.tensor_mul(v11[:, 0:1024], v10[:], s_B[:, 768:1792]),
        lambda: nc.vector.tensor_sub(v11[:, 1024:2048], v10[:], v11[:, 0:1024]),
    ]
    ops += _out_ops(nc, ot, rows, v11, s_B, 1792, fine_tail)
    if defer:
        return ops
    for f in ops:
        f()


def _out_ops(nc, ot, rows, v11, s_B, s0, fine_tail):
    """Level-11 output ops (+ DMAs) as closures; s0 = s_B offset of level 11."""
    if fine_tail:
        # 1024-col pieces, DMA'd as soon as each is ready: the kernel's last
        # DMA shrinks from 1456ns (after the last DVE op) to 728ns.
        return [
            lambda: nc.vector.tensor_mul(ot[:, 0:1024], v11[:, 0:1024],
                                         s_B[:, s0:s0 + 1024]),
            lambda: nc.sync.dma_start(out=rows[:, 0:1024], in_=ot[:, 0:1024]),
            lambda: nc.vector.tensor_mul(ot[:, 1024:2048], v11[:, 1024:2048],
                                         s_B[:, s0 + 1024:s0 + 2048]),
            lambda: nc.sync.dma_start(out=rows[:, 1024:2048], in_=ot[:, 1024:2048]),
            lambda: nc.vector.tensor_sub(ot[:, 2048:3072], v11[:, 0:1024],
                                         ot[:, 0:1024]),
            lambda: nc.sync.dma_start(out=rows[:, 2048:3072], in_=ot[:, 2048:3072]),
            lambda: nc.vector.tensor_sub(ot[:, 3072:4096], v11[:, 1024:2048],
                                         ot[:, 1024:2048]),
            lambda: nc.sync.dma_start(out=rows[:, 3072:4096], in_=ot[:, 3072:4096]),
        ]
    return [
        lambda: nc.vector.tensor_mul(ot[:, 0:2048], v11[:], s_B[:, s0:s0 + 2048]),
        lambda: nc.sync.dma_start(out=rows[:, 0:2048], in_=ot[:, 0:2048]),
        lambda: nc.vector.tensor_sub(ot[:, 2048:4096], v11[:], ot[:, 0:2048]),
        lambda: nc.sync.dma_start(out=rows[:, 2048:4096], in_=ot[:, 2048:4096]),
    ]


def _interleave(a, b, head=3):
    """Emit closure list a, zipping b's ops in after a head start."""
    for f in a[:head]:
        f()
    ia, ib = head, 0
    while ia < len(a) or ib < len(b):
        if ia < len(a):
            a[ia]()
            ia += 1
        if ib < len(b):
            b[ib]()
            ib += 1


def _emit_body(nc, tc, pa, pb, psp, wdt_r, xt_r, out, warm=None):
    SIG = mybir.ActivationFunctionType.Sigmoid
    # half 1 (tiles 0-3) cuts at level 8 to keep the ramp-up chain short;
    # half 2 (tiles 4-7), whose phase A runs mid-stream off the critical
    # path, cuts at level 9 (fewer, larger DVE ops).
    s_A = pa.tile([128, NBT, 512], F16, tag="sA")
    vA = pa.tile([128, NBT, 512], F16, tag="vA")
    vB = pa.tile([128, NBT, 512], F16, tag="vB")

    psA1 = _phase_a_mm(nc, psp, wdt_r, xt_r, 0, 256, warm=warm)
    nc.scalar.activation(
        out=s_A[:, 0:4, 0:256], in_=psA1[:, 0:1024], func=SIG,
    )
    v8_h1 = _phase_a_tree(nc, s_A, vA, vB, 0, 8)

    _tile_deep(nc, psp, pb, wdt_r, xt_r, out, v8_h1, 0)
    # phase-A2 matmul+sigma emitted here so its 1.9us sigma queues BEHIND
    # tile 0's sigma chunks on the in-order Act engine.
    psA2 = _phase_a_mm(nc, psp, wdt_r, xt_r, 1, 512)
    nc.scalar.activation(
        out=s_A[:, 4:8, 0:512].rearrange("p g n -> p (g n)"),
        in_=psA2[:], func=SIG,
    )
    _tile_deep(nc, psp, pb, wdt_r, xt_r, out, v8_h1, 1)
    # half-2 shallow tree lands on the DVE queue here, off the ramp-up path;
    # both halves share the ping-pong pair and end in the same buffer.
    v9_h2 = _phase_a_tree(nc, s_A, vA, vB, 1, 9)
    ops2 = _tile_deep(nc, psp, pb, wdt_r, xt_r, out, v8_h1, 2, defer=True)
    ops3 = _tile_deep(nc, psp, pb, wdt_r, xt_r, out, v8_h1, 3, defer=True)
    _interleave(ops2, ops3)
    ops4 = _tile_deep9(nc, psp, pb, wdt_r, xt_r, out, v9_h2, 4, defer=True)
    ops5 = _tile_deep9(nc, psp, pb, wdt_r, xt_r, out, v9_h2, 5, defer=True)
    _interleave(ops4, ops5)
    ops6 = _tile_deep9(nc, psp, pb, wdt_r, xt_r, out, v9_h2, 6, defer=True)
    ops7 = _tile_deep9(nc, psp, pb, wdt_r, xt_r, out, v9_h2, 7,
                       fine_tail=True, defer=True)
    _interleave(ops6, ops7)


PS1B_LO = 511                  # cutoff-9 tiles: levels 9..11 = cols 511..4094
PS2B_LO = 2559


def _tile_deep9(nc, psp, pb, wdt_r, xt_r, out, v9b, bt, fine_tail=False,
                defer=False):
    """Per batch tile, cutoff-9 variant: matmul cols 511..4094, sigmoid,
    levels 9..11, DMA. Used for tiles whose level-8 was batched in phase A."""
    SIG = mybir.ActivationFunctionType.Sigmoid
    xs = xt_r[:, bt * 128:(bt + 1) * 128]
    ps1 = psp.tile([128, 2048], F32, tag="ps")   # cols 511..2558
    for c in range(4):
        nc.tensor.matmul(
            ps1[:, c * 512:(c + 1) * 512],
            xs, wdt_r[:, PS1B_LO + c * 512:PS1B_LO + (c + 1) * 512],
            start=True, stop=True,
        )
    ps2 = psp.tile([128, 1536], F32, tag="ps")   # cols 2559..4094
    for c in range(3):
        nc.tensor.matmul(
            ps2[:, c * 512:(c + 1) * 512],
            xs, wdt_r[:, PS2B_LO + c * 512:PS2B_LO + (c + 1) * 512],
            start=True, stop=True,
        )
    s_B = pb.tile([128, 3584], F16, tag="sB")    # sigma of cols 511..4094
    nc.scalar.activation(out=s_B[:, 0:512], in_=ps1[:, 0:512], func=SIG)
    nc.scalar.activation(out=s_B[:, 512:2048], in_=ps1[:, 512:2048], func=SIG)
    nc.scalar.activation(out=s_B[:, 2048:3584], in_=ps2[:], func=SIG)

    v10 = pb.tile([128, 1024], F16, tag="v10")
    v11 = pb.tile([128, 2048], F16, tag="v11")
    ot = pb.tile([128, 4096], F16, tag="out")
    rows = out[bt * 128:(bt + 1) * 128, :]
    ops = [
        lambda: nc.vector.tensor_mul(v10[:, 0:512], v9b[:, bt, 0:512],
                                     s_B[:, 0:512]),
        lambda: nc.vector.tensor_sub(v10[:, 512:1024], v9b[:, bt, 0:512],
                                     v10[:, 0:512]),
        lambda: nc.vector.tensor_mul(v11[:, 0:1024], v10[:], s_B[:, 512:1536]),
        lambda: nc.vector.tensor_sub(v11[:, 1024:2048], v10[:], v11[:, 0:1024]),
    ]
    ops += _out_ops(nc, ot, rows, v11, s_B, 1536, fine_tail)
    if defer:
        return ops
    for f in ops:
        f()


_NC_CACHE = {}


def _get_nc(reps=1):
    if reps not in _NC_CACHE:
        _NC_CACHE[reps] = _build(reps)
    return _NC_CACHE[reps]


def _bitrev(i, bits):
    r = 0
    for _ in range(bits):
        r = (r << 1) | (i & 1)
        i >>= 1
    return r


def _node_perm():
    """ours-col -> tree level-order node, per the concat-layout position map.

    Position i at level l corresponds to path bits b_0..b_{l-1} with b_j at
    bit j of i; the level-order node index uses b_0 as MSB -> bitrev_l(i).
    """
    perm = np.empty(NODES, dtype=np.int64)
    for lvl in range(DEPTH):
        off = (1 << lvl) - 1
        for i in range(1 << lvl):
            perm[off + i] = off + _bitrev(i, lvl)
    return perm


_NODE_PERM = _node_perm()
# leaf L lives at raw position bitrev12(L)
_LEAF_PERM = np.array([_bitrev(j, DEPTH) for j in range(LEAVES)], dtype=np.int64)


def _prep_inputs(x, W, b):
    x = np.asarray(x, dtype=np.float32)
    W = np.asarray(W, dtype=np.float32)
    b = np.asarray(b, dtype=np.float32)
    Wd = (W[:, 0, :] - W[:, 1, :])[_NODE_PERM]   # [4095, 64] in ours-col order
    bd = (b[:, 0] - b[:, 1])[_NODE_PERM]         # [4095]
    wdt = np.zeros((KA, LEAVES), dtype=np.float32)
    wdt[:D, :NODES] = Wd.T
    wdt[D, :NODES] = bd
    xt = np.empty((KA, B), dtype=np.float32)
    xt[:D] = x.T
    xt[D] = 1.0
    in_maps = [
        {"wdt": wdt, "xt": np.ascontiguousarray(xt[:, c * BLOC:(c + 1) * BLOC])}
        for c in range(NCORES)
    ]
    return in_maps


def kernel(x, W, b):
    in_maps = _prep_inputs(x, W, b)
    nc = _get_nc()
    res = run_bass_kernel_spmd(nc, in_maps, core_ids=list(range(NCORES)))
    raw = np.concatenate([res.results[c]["out"] for c in range(NCORES)], axis=0)
    return raw[:, _LEAF_PERM].astype(np.float32) * np.float32(1.0 / C_SCALE)


if __name__ == "__main__":
    rng = np.random.default_rng(0)
    x = rng.standard_normal((B, D)).astype(np.float32)
    W = (rng.standard_normal((NODES, 2, D)) * 0.1).astype(np.float32)
    b = (rng.standard_normal((NODES, 2)) * 0.1).astype(np.float32)
    p = kernel(x, W, b)
    print("out", p.shape, p.dtype, "rowsum", p.sum(axis=1)[:4])
